# revision 44
# baseline (speedup 1.0000x reference)
"""Trainium2 8-core kernel for nn_Attention_21345987461594.

Multi-head attention: B=2, S=4096, E=512, H=8 heads, D=64.
  qkv = x @ w_qkv + b_qkv ; per-head softmax(q k^T / sqrt(D)) v ; out proj.

Sharding: 16 (batch, head) pairs -> 2 heads per core (core c: batch c//4,
heads 2*(c%4), 2*(c%4)+1). No collectives: each core computes a partial
out-projection (rows of w_out for its heads) and the host sums the 4
partials per batch. All matmuls run in bf16 (f32 PSUM accumulate);
softmax skips max-subtraction (scores ~ N(0,1) after 1/sqrt(D) scaling,
exp is safely bounded) and the denominator is fused into the PV matmul
as an extra all-ones column of V.

v2 changes vs the 324us baseline:
- ~1/3 of the exp work moves off the (bottleneck) ACT engine: the third
  chunk of each 3-chunk score group is exponentiated on DVE/Pool with a
  single fused tensor_scalar op computing Schraudolph's bit-trick exp
  directly in bf16: int16(x*A + B) reinterpreted as bf16 (A folds the
  1/sqrt(D)*log2(e) scale into the bf16 exponent field). ACT groups
  shrink from [128,1536] to [128,1024], so ACT (~34us/block) drops below
  the PE's ~31us/block and stops pacing the kernel.
- Blocks 1..7 process head 0's 32 key chunks first, then head 1's
  (h-major), so head 0's softmax tail (dn extract -> DRAM-bounce
  broadcast -> reciprocal -> multiply) runs mid-block on idle engines.
  PV accumulators are per-head single-buffer PSUM tiles released by a
  direct PSUM->SBUF multiply (no drain copies), freeing 2 PSUM banks.
- Faster start (k-projection inputs DMA'd first) and a shortened final
  tail (ones-column matmul broadcast for the last head's denominator,
  out-projection immediately after).
"""

import sys

if "/opt/trn_rl_repo" not in sys.path:
    sys.path.insert(0, "/opt/trn_rl_repo")

import numpy as np
import ml_dtypes

import concourse.bass as bass
import concourse.tile as tile
from concourse import bacc, mybir
from concourse.bass_utils import run_bass_kernel_spmd
from concourse.masks import make_identity

BF16 = mybir.dt.bfloat16
I16 = mybir.dt.int16
F32 = mybir.dt.float32

B, S, E, H = 2, 4096, 512, 8
D = E // H          # 64
HPC = 2             # heads per core
N_CORES = 8
QB = 512            # query block (free dim of score matmuls)
N_QB = S // QB      # 8
CH = 128            # key chunk
N_CH = S // CH      # 32
GRP = 3             # score chunks per group (2 on ACT + 1 offloaded)

# Schraudolph bf16 exp: bf16_bits(exp(s/sqrt(D))) ~= int16(s*SCH_A + SCH_B)
SCH_C = 7.4
SCH_A = 128.0 * float(np.log2(np.e)) / 8.0
SCH_B = 16256.0 - SCH_C + 0.5

# fused [V | 1] stationary layout: per key-chunk, 65 cols per head
VW = HPC * (D + 1)  # 130

FW = HPC * D        # 128, qkv projection tile width per ft


def _build():
    nc = bacc.Bacc("TRN2", target_bir_lowering=False)

    xt_ext = nc.declare_dram_parameter("xt", [E, S], BF16, isOutput=False)
    # host-packed partition-major: wqkv[p, e*384 + ft*128 + j] = w[e*128+p, ft*128+j]
    wqkv_ext = nc.declare_dram_parameter("wqkv", [128, 12 * FW], BF16, isOutput=False)
    bqkv_ext = nc.declare_dram_parameter("bqkv", [128, 3], F32, isOutput=False)
    wout_ext = nc.declare_dram_parameter("wout", [FW, E], BF16, isOutput=False)
    out_ext = nc.declare_dram_parameter("out", [E, S], BF16, isOutput=True)
    # DRAM bounce for the softmax-reciprocal partition broadcast
    dn_scr = [nc.dram_tensor(f"dnscr{i}", [HPC, QB], F32) for i in range(2)]

    with tile.TileContext(nc) as tc:
        with (
            tc.tile_pool(name="consts", bufs=1) as consts,
            tc.tile_pool(name="pta_pool", bufs=11) as pta_pool,
            tc.tile_pool(name="ptb_pool", bufs=11) as ptb_pool,
            tc.tile_pool(name="attn_pool", bufs=2) as attn_pool,
            tc.tile_pool(name="ot_pool", bufs=4) as ot_pool,
            tc.tile_pool(name="sm_pool", bufs=2) as sm_pool,
            tc.tile_pool(name="psum_sc", bufs=2, space="PSUM") as psum_sc,
            tc.tile_pool(name="psum_off", bufs=2, space="PSUM") as psum_off,
            tc.tile_pool(name="psum_pv0", bufs=1, space="PSUM") as psum_pv0,
            tc.tile_pool(name="psum_pv1", bufs=1, space="PSUM") as psum_pv1,
        ):
            pv_pools = (psum_pv0, psum_pv1)
            # ---- persistent SBUF tensors ----
            # packed layouts: one strided DMA loads all 4 e-chunks of a
            # token-block (or of a w_qkv ft-slice) at once.
            xt_all = consts.tile([128, 4 * S], BF16, name="xt")
            xt_view = xt_all.rearrange("p (e t) -> p e t", e=4)
            wq_all = consts.tile([128, 12 * FW], BF16, name="wqall")
            wout_sb = consts.tile([128, E], BF16, name="wout")
            b_all = consts.tile([128, 3], F32, name="ball")
            qT = consts.tile([128, S], BF16, name="qT")
            kT = consts.tile([128, S], BF16, name="kT")
            vT = consts.tile([128, S], BF16, name="vT")
            V_sb = consts.tile([128, N_CH * VW], BF16, name="V")
            ident_bf = consts.tile([128, 128], BF16, name="ident")
            ones_col = consts.tile([1, D], BF16, name="ones_col")

            # ---- loads / constants (critical path first: k-proj ft=1) ----
            _xt_base = xt_ext[:, :]

            def xt_src(tb):
                return bass.AP(
                    tensor=_xt_base.tensor,
                    offset=_xt_base.offset + tb * QB,
                    ap=[[S, 128], [128 * S, 4], [1, QB]],
                )

            def xt_dst(tb):
                return xt_view[:, :, tb * QB : (tb + 1) * QB]

            def xt_src_half(tb, e0):
                return bass.AP(
                    tensor=_xt_base.tensor,
                    offset=_xt_base.offset + e0 * 128 * S + tb * QB,
                    ap=[[S, 128], [128 * S, 2], [1, QB]],
                )

            # host-packed wq: one fully contiguous DMA on the scalar queue
            nc.scalar.dma_start(out=wq_all, in_=wqkv_ext[:, :])
            # first token-block split across both queues (critical path)
            nc.sync.dma_start(out=xt_view[:, 0:2, 0:QB], in_=xt_src_half(0, 0))
            nc.scalar.dma_start(out=xt_view[:, 2:4, 0:QB], in_=xt_src_half(0, 2))
            nc.scalar.dma_start(out=b_all, in_=bqkv_ext[:, :])
            nc.sync.dma_start(out=xt_dst(1), in_=xt_src(1))
            nc.scalar.dma_start(out=wout_sb, in_=wout_ext[:, :])
            make_identity(nc, ident_bf)
            V_view = V_sb.rearrange("p (c w) -> p c w", w=VW)
            nc.vector.memset(ones_col, 1.0)
            nc.vector.memset(V_view[:, :, D : D + 1], 1.0)
            nc.vector.memset(V_view[:, :, VW - 1 : VW], 1.0)
            # bulk xt loads go on the Sync HWDGE queue only: DMAs issued from
            # nc.scalar occupy the ACT sequencer and would delay the first exps
            for tb in range(2, N_QB):
                nc.sync.dma_start(out=xt_dst(tb), in_=xt_src(tb))

            # ---- qkv projection: (q|k|v)^T[f, t] ----
            dests = (qT, kT, vT)

            def proj(ft, tbs):
                assert len(tbs) <= 2
                ps = psum_sc.tile(
                    [128, 2 * QB], F32, tag="sc", name=f"prj{ft}_{tbs[0]}"
                )
                for e in range(4):
                    for i, tb in enumerate(tbs):
                        nc.tensor.matmul(
                            ps[:, i * QB : (i + 1) * QB],
                            lhsT=wq_all[:, (3 * e + ft) * FW : (3 * e + ft + 1) * FW],
                            rhs=xt_view[:, e, tb * QB : (tb + 1) * QB],
                            start=(e == 0),
                            stop=(e == 3),
                        )
                for i, tb in enumerate(tbs):
                    nc.vector.tensor_scalar_add(
                        out=dests[ft][:, tb * QB : (tb + 1) * QB],
                        in0=ps[:, i * QB : (i + 1) * QB],
                        scalar1=b_all[:, ft : ft + 1],
                    )

            def vbuild(c0):
                tp = psum_sc.tile([128, 2 * QB], BF16, tag="sc", name=f"tp{c0}")
                for i in range(2):
                    c = c0 + i
                    nc.tensor.transpose(
                        tp[:, i * 128 : (i + 1) * 128],
                        vT[:, c * 128 : (c + 1) * 128],
                        ident_bf,
                    )
                for i in range(2):
                    c = c0 + i
                    nc.vector.tensor_copy(
                        out=V_view[:, c, 0:D], in_=tp[:, i * 128 : i * 128 + D]
                    )
                    nc.vector.tensor_copy(
                        out=V_view[:, c, D + 1 : VW - 1],
                        in_=tp[:, i * 128 + D : i * 128 + 2 * D],
                    )

            # upfront: k/q projections for block 0 only; the rest drips into
            # the attention stream.
            proj(1, [0])
            proj(0, [0])
            extras = [
                (proj, 1, [1]), (proj, 2, [0]), (vbuild, 0), (vbuild, 2),
                (proj, 1, [2]), (proj, 2, [1]), (vbuild, 4), (vbuild, 6),
                (proj, 1, [3]), (proj, 2, [2]), (vbuild, 8), (vbuild, 10),
                (proj, 1, [4]), (proj, 2, [3]), (vbuild, 12), (vbuild, 14),
                (proj, 1, [5]), (proj, 0, [1]), (proj, 2, [4]),
                (vbuild, 16), (vbuild, 18),
                (proj, 1, [6]), (proj, 2, [5]), (vbuild, 20), (vbuild, 22),
                (proj, 1, [7]), (proj, 2, [6]), (vbuild, 24), (vbuild, 26),
                (proj, 2, [7]), (vbuild, 28), (vbuild, 30),
                (proj, 0, [2]), (proj, 0, [3]), (proj, 0, [4]),
                (proj, 0, [5]), (proj, 0, [6]), (proj, 0, [7]),
            ]

            # ---- attention group structure ----
            # block 0: 22 groups over interleaved m-chunks (m = 2c+h), v1
            # style: g0 = 1 chunk, g1..21 = 3 chunks (last one offloaded).
            # blocks 1..7: per head h: g0 = chunks {0,1} (ACT), g1..g10 =
            # {3g-1, 3g} (ACT) + {3g+1} (offloaded).
            def group_chunks(qb, gi):
                """-> (head_or_None, [(c, h), ...]) chunk list; last entry is
                the offloaded one iff the group has 3 chunks."""
                if qb == 0:
                    if gi == 0:
                        ms = [0]
                    else:
                        ms = [3 * gi - 2, 3 * gi - 1, 3 * gi]
                    return [(m >> 1, m & 1) for m in ms]
                h, g = divmod(gi, 11)
                if g == 0:
                    cs = [0, 1]
                else:
                    cs = [3 * g - 1, 3 * g, 3 * g + 1]
                return [(c, h) for c in cs]

            N_GRP = 22  # groups per block (both layouts)

            def emit_scores_exp(st, gi):
                qb = st["qb"]
                chunks = group_chunks(qb, gi)
                n = len(chunks)
                n_act = min(n, 2)
                # the first two chunks and the offloaded third live in
                # separate PSUM pools and separate pt tiles: the ACT exp and
                # the DVE Schraudolph exp must share NO tile, or a
                # write-after-write dependency serializes them and the
                # exp->offload->psum-free->scores->exp cycle paces the kernel.
                sc = psum_sc.tile([128, 2 * QB], F32, tag="sc", name=f"sc{qb}_{gi}")
                pta = pta_pool.tile(
                    [128, 2 * QB], BF16, tag="pta", name=f"pta{qb}_{gi}"
                )
                ptb = None
                for s, (c, h) in enumerate(chunks[:2]):
                    nc.tensor.matmul(
                        sc[:, s * QB : (s + 1) * QB],
                        lhsT=kT[h * D : (h + 1) * D, c * CH : (c + 1) * CH],
                        rhs=qT[h * D : (h + 1) * D, qb * QB : (qb + 1) * QB],
                        start=True,
                        stop=True,
                    )
                # the ACT exp is emitted BEFORE the offloaded chunk's score
                # matmul: its PE-counter wait threshold must not cover the
                # sco matmul, or the compiler strength-reduces the DVE
                # offload's wait into "after exp(g)" and serializes the two
                # exp engines.
                nc.scalar.activation(
                    out=pta[:, : n_act * QB],
                    in_=sc[:, : n_act * QB],
                    func=mybir.ActivationFunctionType.Exp,
                    scale=float(D) ** -0.5,
                )
                if n > 2:
                    c, h = chunks[2]
                    sco = psum_off.tile([128, QB], F32, tag="off", name=f"sco{qb}_{gi}")
                    nc.tensor.matmul(
                        sco[:, :],
                        lhsT=kT[h * D : (h + 1) * D, c * CH : (c + 1) * CH],
                        rhs=qT[h * D : (h + 1) * D, qb * QB : (qb + 1) * QB],
                        start=True,
                        stop=True,
                    )
                    ptb = ptb_pool.tile([128, QB], BF16, tag="ptb", name=f"ptb{qb}_{gi}")
                    if qb == 0:
                        # block 0: the DVE is saturated with qkv bias adds
                        # and V-build copies; offloading there causes mutual
                        # PE/DVE stalls. Use a second ACT activation instead.
                        nc.scalar.activation(
                            out=ptb[:, :],
                            in_=sco[:, :],
                            func=mybir.ActivationFunctionType.Exp,
                            scale=float(D) ** -0.5,
                        )
                    else:
                        # Pool cannot read PSUM on TRN2; all offloaded exps
                        # run on the DVE.
                        nc.vector.tensor_scalar(
                            out=ptb[:, :].bitcast(I16),
                            in0=sco[:, :],
                            scalar1=SCH_A,
                            scalar2=SCH_B,
                            op0=mybir.AluOpType.mult,
                            op1=mybir.AluOpType.add,
                        )
                st["pts"][gi] = (pta, ptb)

            def emit_pv(st, gi):
                qb = st["qb"]
                chunks = group_chunks(qb, gi)
                pta, ptb = st["pts"].pop(gi)
                for s, (c, h) in enumerate(chunks):
                    if st["pv"][h] is None:
                        st["pv"][h] = pv_pools[h].tile(
                            [128, QB], F32, tag=f"pv{h}", name=f"pv{qb}_{h}"
                        )
                    rhs = ptb[:, 0:QB] if s == 2 else pta[:, s * QB : (s + 1) * QB]
                    nc.tensor.matmul(
                        st["pv"][h][0 : D + 1, :],
                        lhsT=V_sb[:, c * VW + h * (D + 1) : c * VW + (h + 1) * (D + 1)],
                        rhs=rhs,
                        start=(c == 0),
                        stop=(c == N_CH - 1),
                    )

            # ---- tails ----
            def tail_b0(st, step):
                # block 0 (interleaved heads): copy-drain both heads fast to
                # free the PSUM accumulators, then the v1 softmax tail.
                qb = st["qb"]
                if step == 0:
                    st["pvsb2"] = sm_pool.tile(
                        [128, QB], F32, tag="pvsb2", bufs=1, name="pvsb2_0"
                    )
                    st["dn"] = [
                        sm_pool.tile([1, QB], F32, tag=f"dn{h}", bufs=2, name=f"dn0_{h}")
                        for h in range(HPC)
                    ]
                    nc.vector.tensor_copy(
                        out=st["pvsb2"][0:D, :], in_=st["pv"][0][0:D, :]
                    )
                    nc.scalar.copy(
                        out=st["pvsb2"][D : 2 * D, :], in_=st["pv"][1][0:D, :]
                    )
                    nc.vector.tensor_copy(out=st["dn"][0], in_=st["pv"][0][D : D + 1, :])
                    nc.vector.tensor_copy(out=st["dn"][1], in_=st["pv"][1][D : D + 1, :])
                elif step == 1:
                    for h in range(HPC):
                        nc.sync.dma_start(
                            out=dn_scr[0][h : h + 1, :], in_=st["dn"][h]
                        )
                elif step == 2:
                    st["dnb"] = sm_pool.tile(
                        [128, QB], F32, tag="dnbf0", bufs=1, name="dnb0"
                    )
                    for h in range(HPC):
                        row = dn_scr[0][h : h + 1, :]
                        src = bass.AP(
                            tensor=row.tensor,
                            offset=row.offset,
                            ap=[[0, D]] + list(row.ap),
                        )
                        nc.gpsimd.dma_start(
                            out=st["dnb"][h * D : (h + 1) * D, :], in_=src
                        )
                elif step == 3:
                    st["rcp"] = sm_pool.tile(
                        [128, QB], F32, tag="rcpf0", bufs=1, name="rcp0"
                    )
                    nc.vector.reciprocal_approx_fast(out=st["rcp"], in_=st["dnb"])
                elif step == 4:
                    st["attnT"] = attn_pool.tile(
                        [128, QB], BF16, tag="attnT", name=f"attnT{qb}"
                    )
                    nc.vector.tensor_mul(
                        out=st["attnT"], in0=st["pvsb2"], in1=st["rcp"]
                    )
                elif step in (5, 6):
                    op_mm(st, step - 5)
                else:
                    op_cast(st, step - 7)

            def op_mm(st, pair):
                op = psum_sc.tile(
                    [128, 2 * QB], F32, tag="sc", name=f"op{st['qb']}_{pair}"
                )
                st["op"][pair] = op
                for k in range(2):
                    et = pair * 2 + k
                    nc.tensor.matmul(
                        op[:, k * QB : (k + 1) * QB],
                        lhsT=wout_sb[:, et * 128 : (et + 1) * 128],
                        rhs=st["attnT"],
                        start=True,
                        stop=True,
                    )

            def op_cast(st, pair):
                # casts run 1+ slot after the matmuls: emitted any earlier
                # they stall their engine's in-order queue on the PE, and the
                # released PSUM tile gates the score ring
                qb = st["qb"]
                op = st["op"][pair]
                for k in range(2):
                    et = pair * 2 + k
                    ot = ot_pool.tile([128, QB], BF16, tag="ot")
                    if k == 1:
                        nc.scalar.copy(out=ot, in_=op[:, k * QB : (k + 1) * QB])
                    else:
                        nc.vector.tensor_copy(out=ot, in_=op[:, k * QB : (k + 1) * QB])
                    # last block: spread the final DMAs over three queues
                    if qb == N_QB - 1:
                        dq = (nc.sync, nc.gpsimd, nc.scalar, nc.sync)[et]
                    else:
                        dq = nc.sync
                    dq.dma_start(
                        out=out_ext[et * 128 : (et + 1) * 128, qb * QB : (qb + 1) * QB],
                        in_=ot,
                    )

            def tail_h(st, h, step):
                # blocks 1..7, per-head tail; h0 runs mid-block, h1 spills
                # into the next block.
                qb = st["qb"]
                if step == 0:
                    st["dn"][h] = sm_pool.tile(
                        [1, QB], F32, tag=f"dn{h}", bufs=2, name=f"dn{qb}_{h}"
                    )
                    nc.vector.tensor_copy(
                        out=st["dn"][h], in_=st["pv"][h][D : D + 1, :]
                    )
                elif step == 1:
                    nc.sync.dma_start(
                        out=dn_scr[qb % 2][h : h + 1, :], in_=st["dn"][h]
                    )
                elif step == 2:
                    st["dnb"][h] = sm_pool.tile(
                        [D, QB], F32, tag=f"dnb{h}", bufs=2, name=f"dnb{qb}_{h}"
                    )
                    row = dn_scr[qb % 2][h : h + 1, :]
                    src = bass.AP(
                        tensor=row.tensor,
                        offset=row.offset,
                        ap=[[0, D]] + list(row.ap),
                    )
                    nc.gpsimd.dma_start(out=st["dnb"][h], in_=src)
                elif step == 3:
                    st["rcp"][h] = sm_pool.tile(
                        [D, QB], F32, tag=f"rcp{h}", bufs=2, name=f"rcp{qb}_{h}"
                    )
                    nc.vector.reciprocal_approx_fast(
                        out=st["rcp"][h], in_=st["dnb"][h]
                    )
                elif step == 4:
                    if st["attnT"] is None:
                        st["attnT"] = attn_pool.tile(
                            [128, QB], BF16, tag="attnT", name=f"attnT{qb}"
                        )
                    nc.vector.tensor_mul(
                        out=st["attnT"][h * D : (h + 1) * D, :],
                        in0=st["pv"][h][0:D, :],
                        in1=st["rcp"][h],
                    )
                elif step in (5, 6):
                    op_mm(st, step - 5)
                else:
                    op_cast(st, step - 7)

            def tail_fast(st, h, step):
                # last block, head 1: ones-matmul broadcast, everything ASAP.
                qb = st["qb"]
                if step == 0:
                    # split the single-partition dn copy across DVE and ACT:
                    # halves run in parallel on the exposed final tail
                    st["dnbf"] = sm_pool.tile(
                        [1, QB], BF16, tag="dnbf", bufs=1, name="dnbf7"
                    )
                    nc.vector.tensor_copy(
                        out=st["dnbf"][:, 0 : QB // 2],
                        in_=st["pv"][1][D : D + 1, 0 : QB // 2],
                    )
                    nc.scalar.copy(
                        out=st["dnbf"][:, QB // 2 : QB],
                        in_=st["pv"][1][D : D + 1, QB // 2 : QB],
                    )
                elif step == 1:
                    st["rcb"] = psum_sc.tile(
                        [128, 2 * QB], F32, tag="sc", name="rcb7"
                    )
                    nc.tensor.matmul(
                        st["rcb"][0:D, 0:QB],
                        lhsT=ones_col,
                        rhs=st["dnbf"],
                        start=True,
                        stop=True,
                    )
                elif step == 2:
                    st["rcp"][1] = sm_pool.tile(
                        [D, QB], F32, tag="rcp1", bufs=2, name="rcp7_1"
                    )
                    nc.vector.reciprocal_approx_fast(
                        out=st["rcp"][1], in_=st["rcb"][0:D, 0:QB]
                    )
                elif step == 3:
                    nc.vector.tensor_mul(
                        out=st["attnT"][D : 2 * D, :],
                        in0=st["pv"][1][0:D, :],
                        in1=st["rcp"][1],
                    )
                elif step in (4, 5):
                    op_mm(st, step - 4)
                else:
                    op_cast(st, step - 6)

            # ---- slot scheduler ----
            B0_TAIL_OFFS = (0, 1, 2, 4, 6, 8, 9, 10, 11)
            H_TAIL_OFFS = (1, 2, 3, 5, 6)        # per-head steps 0..4
            OP_OFFS = (8, 9, 10, 11)             # op mm pairs, then casts
            F_TAIL_OFFS = (0, 1, 2, 3, 4, 5, 6, 7)  # last-block h1 fast tail

            slot = 0
            pvq = []     # pending (st, gi, emit_slot)
            tails = []   # (fn, args, due_slot)
            done = {"k": 0, "q": 0, "vb": 0}
            outstanding = [0]  # emitted-not-consumed pt groups

            def pop_extra():
                fn, *args = extras.pop(0)
                fn(*args)
                if fn is proj:
                    if args[0] == 1:
                        done["k"] = max(done["k"], max(args[1]))
                    elif args[0] == 0:
                        done["q"] = max(done["q"], max(args[1]))
                else:
                    done["vb"] += 2

            def vb_need(st, gi):
                chunks = group_chunks(st["qb"], gi)
                return min(N_CH, max(c for c, h in chunks) + 1)

            def pump_pv(limit=2):
                for _ in range(limit):
                    if not pvq:
                        return
                    s2, g2, es = pvq[0]
                    if slot < es + 2:
                        return
                    if done["vb"] < vb_need(s2, g2):
                        if extras:
                            pop_extra()
                            continue
                        return
                    pvq.pop(0)
                    outstanding[0] -= 1
                    emit_pv(s2, g2)
                    qb = s2["qb"]
                    if qb == 0:
                        if g2 == N_GRP - 1:
                            for k, off in enumerate(B0_TAIL_OFFS):
                                tails.append((tail_b0, (s2, k), slot + off))
                            return
                    else:
                        h, g = divmod(g2, 11)
                        if g == 10:
                            last = qb == N_QB - 1
                            if last and h == 1:
                                for k, off in enumerate(F_TAIL_OFFS):
                                    tails.append((tail_fast, (s2, 1, k), slot + off))
                            else:
                                for k, off in enumerate(H_TAIL_OFFS):
                                    tails.append((tail_h, (s2, h, k), slot + off))
                                if h == 1:
                                    for k, off in enumerate(OP_OFFS):
                                        tails.append(
                                            (tail_h, (s2, 1, 5 + k), slot + off)
                                        )
                            return

            def pump_tails():
                while tails and tails[0][2] <= slot:
                    fn, args, _ = tails.pop(0)
                    fn(*args)

            for qb in range(N_QB):
                st = {
                    "qb": qb, "pts": {}, "pv": [None, None], "dn": [None, None],
                    "dnb": [None, None], "rcp": [None, None], "attnT": None,
                    "op": [None, None],
                }
                if qb == 0:
                    st["attnT"] = None
                for gi in range(N_GRP):
                    if qb == 0:
                        k_need = min(N_QB - 1, (3 * gi + 2) // 8)
                    else:
                        h, g = divmod(gi, 11)
                        c1 = 1 if g == 0 else 3 * g + 1
                        k_need = min(N_QB - 1, c1 // 4)
                    while extras and (done["k"] < k_need or done["q"] < qb):
                        pop_extra()
                    # back-pressure: never let exp production run more than
                    # 9 groups ahead of PV consumption (pt pools are 12 deep;
                    # the PE executes in emission order, so a stalled score
                    # matmul ahead of pending PV work would deadlock).
                    while outstanding[0] >= 9:
                        pump_pv()
                    emit_scores_exp(st, gi)
                    pvq.append((st, gi, slot))
                    outstanding[0] += 1
                    if extras:
                        pop_extra()
                    # skip PV pops on the slot right before a block boundary
                    # so the next block's score matmuls issue immediately
                    if gi != N_GRP - 1:
                        pump_pv()
                    pump_tails()
                    slot += 1
            while extras:
                pop_extra()
            while pvq or tails:
                pump_pv()
                pump_tails()
                slot += 1

    nc.compile()
    return nc


_NC = None
LAST = {}


def _get_nc():
    global _NC
    if _NC is None:
        _NC = _build()
    return _NC


def kernel(x, w_qkv, b_qkv, w_out, b_out):
    x = np.asarray(x, dtype=np.float32)
    w_qkv = np.asarray(w_qkv, dtype=np.float32)
    b_qkv = np.asarray(b_qkv, dtype=np.float32)
    w_out = np.asarray(w_out, dtype=np.float32)
    b_out = np.asarray(b_out, dtype=np.float32)

    bf = ml_dtypes.bfloat16
    in_maps = []
    for c in range(N_CORES):
        b = c // 4
        h0 = (c % 4) * HPC * D  # first head's column offset (2 heads = 128 cols)
        w_slice = np.concatenate(
            [w_qkv[:, j * E + h0 : j * E + h0 + HPC * D] for j in range(3)], axis=1
        )
        # partition-major packing: [128, e*384 + ft*128 + j] = w[e*128+p, ...]
        w_packed = w_slice.reshape(4, 128, 3 * HPC * D).transpose(1, 0, 2).reshape(
            128, 12 * HPC * D
        )
        b_slice = np.stack(
            [
                b_qkv[j * E + h0 : j * E + h0 + HPC * D].astype(np.float32)
                for j in range(3)
            ],
            axis=1,
        )  # [128, 3]
        in_maps.append(
            {
                "xt": np.ascontiguousarray(x[b].T).astype(bf),
                "wqkv": np.ascontiguousarray(w_packed).astype(bf),
                "bqkv": np.ascontiguousarray(b_slice),
                "wout": np.ascontiguousarray(w_out[h0 : h0 + HPC * D, :]).astype(bf),
            }
        )

    res = run_bass_kernel_spmd(_get_nc(), in_maps, list(range(N_CORES)))
    LAST["exec_time_ns"] = res.exec_time_ns
    LAST["res"] = res

    out = np.empty((B, S, E), dtype=np.float32)
    for b in range(B):
        acc = res.results[4 * b]["out"].astype(np.float32)
        for c in range(4 * b + 1, 4 * b + 4):
            acc = acc + res.results[c]["out"]
        out[b] = acc.T + b_out[None, :]
    return out
